# revision 1
# baseline (speedup 1.0000x reference)
"""ChebConv (K=2) + temporal Conv1d GNN kernel for 8 Trainium2 NeuronCores.

Strategy (data-parallel over destination nodes):
  - Node axis padded to 50176 = 392 blocks of 128; core c owns blocks
    [49c, 49c+49).
  - Host precomputes w_hat (edge weights of -D^-1/2 A D^-1/2) and sorts the
    edge list by (dst block, src half, dst subblock-of-32), padding each
    group to a multiple of 128 so all 8 cores share one static program.
  - Per block, the device gathers x rows of the edges' sources from an
    fp16 node-major copy of x via SWDGE dma_gather (two calls: src halves,
    since gather indices are int16), builds a sparse "one-hot * w_hat"
    matrix on the fly with broadcast-AP is_equal/mult, and reduces the
    messages with TensorE matmuls into PSUM (segment-sum as matmul).
  - The Chebyshev combine + temporal conv collapse into dense per-node
    matmuls with host-prefolded weights; LeakyReLU finishes on-chip.
"""

import numpy as np

N = 50000
E = 1600000
W = 12
C = 32
WC = W * C            # 384
NCORES = 8
P = 128
NPAD = 50176          # 392 * 128
NB = NPAD // P        # 392
SLOTS = NB // NCORES  # 49
HALF = NPAD // 2      # 25088
NSB = 4               # dst subblocks of 32 per block

_cache = {}


def _host_prep(x, A, Ew):
    src = np.asarray(A[0], np.int64)
    dst = np.asarray(A[1], np.int64)
    Ew = np.asarray(Ew, np.float32)

    deg = np.bincount(dst, weights=Ew.astype(np.float64), minlength=N).astype(np.float32)
    dinv = np.where(deg > 0, 1.0 / np.sqrt(np.maximum(deg, 1e-12)), 0.0).astype(np.float32)
    w_hat = (-dinv[src] * Ew * dinv[dst]).astype(np.float32)

    # node-major x: [NPAD, W*C]
    xrow = np.zeros((NPAD, WC), np.float32)
    xrow[:N] = np.asarray(x, np.float32).transpose(1, 0, 2).reshape(N, WC)
    xrow16 = xrow.astype(np.float16)

    blk = dst >> 7
    sb = (dst >> 5) & 3
    hh = (src >= HALF).astype(np.int64)
    gid = (blk * 2 + hh) * 4 + sb
    order = np.argsort(gid, kind="stable")
    g_sorted = gid[order]
    src_s = src[order]
    dstl_s = (dst[order] & 31).astype(np.float16)
    what_s = w_hat[order].astype(np.float16)
    counts = np.bincount(gid, minlength=NB * 8).reshape(NB, 2, 4)
    gstart = np.zeros(NB * 8 + 1, np.int64)
    np.cumsum(counts.reshape(-1), out=gstart[1:])

    # static chunk counts per (slot, h, s): max over cores
    cnt_c = counts.reshape(NCORES, SLOTS, 2, 4)
    Kg = np.maximum(1, -(-cnt_c // 128)).max(axis=0)  # [SLOTS, 2, 4]
    Jh = Kg.sum(axis=2)                               # [SLOTS, 2]
    Ji = Jh.sum(axis=1)                               # [SLOTS]
    JT = int(Ji.sum())
    IWT = JT * 8

    # column offsets
    joff = np.zeros(SLOTS + 1, np.int64)
    np.cumsum(Ji, out=joff[1:])
    ioff = joff * 8

    idx16 = np.zeros((NCORES, 128, IWT), np.int16)
    dstl_t = np.zeros((NCORES, 128, JT), np.float16)
    what_t = np.zeros((NCORES, 128, JT), np.float16)
    xslot = np.zeros((NCORES, SLOTS * P, WC), np.float32)

    for c in range(NCORES):
        xslot[c] = xrow[c * SLOTS * P:(c + 1) * SLOTS * P]
        for i in range(SLOTS):
            b = c * SLOTS + i
            for h in range(2):
                L = int(Jh[i, h]) * 128
                V = np.zeros(L, np.int16)
                D = np.zeros(L, np.float16)
                Wv = np.zeros(L, np.float16)
                base = 0
                for s in range(4):
                    g = (b * 2 + h) * 4 + s
                    n = int(gstart[g + 1] - gstart[g])
                    sl = slice(int(gstart[g]), int(gstart[g] + n))
                    V[base:base + n] = (src_s[sl] - h * HALF).astype(np.int16)
                    D[base:base + n] = dstl_s[sl]
                    Wv[base:base + n] = what_s[sl]
                    base += int(Kg[i, h, s]) * 128
                co = int(joff[i] + (Jh[i, 0] if h else 0))
                idx_blk = V.reshape(-1, 16).T                    # [16, L/16]
                idx16[c, :, co * 8: co * 8 + L // 16] = np.tile(idx_blk, (8, 1))
                dstl_t[c, :, co: co + L // 128] = D.reshape(-1, 128).T
                what_t[c, :, co: co + L // 128] = Wv.reshape(-1, 128).T

    return xrow16, xslot, idx16, dstl_t, what_t, Kg, Jh, Ji, joff, JT, IWT


def _fold_weights(Wcheb, bcheb, Wconv, bconv):
    Wcheb = np.asarray(Wcheb, np.float32)
    bcheb = np.asarray(bcheb, np.float32)
    Wconv = np.asarray(Wconv, np.float32)
    bconv = np.asarray(bconv, np.float32)
    # pairs (path, gi, go) with |gi-go|<=1
    pairs = []
    for go in range(3):
        for gi in range(max(0, go - 1), min(3, go + 2)):
            for path in range(2):
                pairs.append((path, gi, go))
    mats = np.zeros((len(pairs), 128, 128), np.float32)
    for pi, (path, gi, go) in enumerate(pairs):
        for wo in range(4 * go, 4 * go + 4):
            for k in range(3):
                wi = wo + k - 1
                if not (4 * gi <= wi < 4 * gi + 4) or not (0 <= wi < W):
                    continue
                Cmat = Wcheb[wi, path] @ Wconv[:, :, k].T  # [ci, co]
                r0 = 32 * (wi - 4 * gi)
                c0 = 32 * (wo - 4 * go)
                mats[pi, r0:r0 + 32, c0:c0 + 32] = Cmat
    mats_sb = np.ascontiguousarray(mats.transpose(1, 0, 2).reshape(128, -1))
    bias = np.zeros((12, 32), np.float32)
    for wo in range(12):
        bias[wo] = bconv.copy()
        for k in range(3):
            wi = wo + k - 1
            if 0 <= wi < W:
                bias[wo] += bcheb[wi] @ Wconv[:, :, k].T
    bias_sb = bias.reshape(3, 128).T.copy()  # [128, 3]
    return mats_sb, bias_sb, pairs


def _build_program(Kg, Jh, Ji, joff, JT, IWT, n_pairs):
    import concourse.bacc as bacc
    import concourse.tile as tile
    from concourse import mybir
    import concourse.bass as bass  # noqa

    nc = bacc.Bacc("TRN2", target_bir_lowering=False, debug=False,
                   num_devices=NCORES)
    f16, f32, i16 = mybir.dt.float16, mybir.dt.float32, mybir.dt.int16
    xrow16 = nc.dram_tensor("xrow16", [NPAD, WC], f16, kind="ExternalInput")
    xslot = nc.dram_tensor("xslot", [SLOTS * P, WC], f32, kind="ExternalInput")
    idx16 = nc.dram_tensor("idx16", [128, IWT], i16, kind="ExternalInput")
    dstl = nc.dram_tensor("dstl", [128, JT], f16, kind="ExternalInput")
    what = nc.dram_tensor("what", [128, JT], f16, kind="ExternalInput")
    mats = nc.dram_tensor("mats", [128, n_pairs * 128], f32, kind="ExternalInput")
    biasd = nc.dram_tensor("biasd", [128, 3], f32, kind="ExternalInput")
    iota = nc.dram_tensor("iota", [128, 32], f16, kind="ExternalInput")
    ident = nc.dram_tensor("ident", [128, 128], f32, kind="ExternalInput")
    out_pc = nc.dram_tensor("out_pc", [SLOTS * P, WC], f32, kind="ExternalOutput")

    pairs_by_go = [[], [], []]
    pi = 0
    for go in range(3):
        for gi in range(max(0, go - 1), min(3, go + 2)):
            for path in range(2):
                pairs_by_go[go].append((pi, gi, path))
                pi += 1

    with tile.TileContext(nc) as tc:
        with tc.tile_pool(name="const", bufs=1) as cp, \
             tc.tile_pool(name="sb", bufs=2) as sb, \
             tc.tile_pool(name="xgp", bufs=2) as xgp, \
             tc.tile_pool(name="pst1", bufs=2, space="PSUM") as pst1, \
             tc.tile_pool(name="pstr", bufs=2, space="PSUM") as pstr, \
             tc.tile_pool(name="psy", bufs=2, space="PSUM") as psy:
            mats_t = cp.tile([128, n_pairs * 128], f32)
            nc.sync.dma_start(out=mats_t[:], in_=mats.ap())
            bias_t = cp.tile([128, 3], f32)
            nc.sync.dma_start(out=bias_t[:], in_=biasd.ap())
            iota_t = cp.tile([128, 32], f16)
            nc.sync.dma_start(out=iota_t[:], in_=iota.ap())
            id_t = cp.tile([128, 128], f32)
            nc.sync.dma_start(out=id_t[:], in_=ident.ap())

            import os
            nslots = int(os.environ.get("K_SLOTS", SLOTS))
            sp_flag = os.environ.get("K_SINGLE_PACKET", "0") == "1"
            JMAX = int(Ji.max())
            for i in range(nslots):
                J0, J1 = int(Jh[i, 0]), int(Jh[i, 1])
                J = J0 + J1
                jo = int(joff[i])

                idx_t = sb.tile([128, JMAX * 8], i16, tag="idx")
                nc.sync.dma_start(out=idx_t[:, :J * 8],
                                  in_=idx16.ap()[:, jo * 8:(jo + J) * 8])
                dm_t = sb.tile([128, JMAX], f16, tag="dm")
                nc.sync.dma_start(out=dm_t[:, :J], in_=dstl.ap()[:, jo:jo + J])
                wh_t = sb.tile([128, JMAX], f16, tag="wh")
                nc.sync.dma_start(out=wh_t[:, :J], in_=what.ap()[:, jo:jo + J])

                xg = xgp.tile([128, JMAX, WC], f16, tag="xg")
                nc.gpsimd.dma_gather(
                    xg[:, 0:J0, :], xrow16.ap()[0:HALF, :],
                    idx_t[:, 0:J0 * 8], J0 * 128, J0 * 128, WC,
                    single_packet=sp_flag)
                nc.gpsimd.dma_gather(
                    xg[:, J0:J, :], xrow16.ap()[HALF:NPAD, :],
                    idx_t[:, J0 * 8:J * 8], J1 * 128, J1 * 128, WC,
                    single_packet=sp_flag)

                eq = sb.tile([128, JMAX, 32], f16, tag="eq")
                nc.vector.tensor_tensor(
                    out=eq[:, :J, :],
                    in0=dm_t[:, :J].unsqueeze(2).to_broadcast([128, J, 32]),
                    in1=iota_t[:].unsqueeze(1).to_broadcast([128, J, 32]),
                    op=mybir.AluOpType.is_equal)
                wm = sb.tile([128, JMAX, 32], f16, tag="wm")
                nc.vector.tensor_tensor(
                    out=wm[:, :J, :],
                    in0=eq[:, :J, :],
                    in1=wh_t[:, :J].unsqueeze(2).to_broadcast([128, J, 32]),
                    op=mybir.AluOpType.mult)

                psum_t1 = pst1.tile([128, WC], f32, space="PSUM", tag="t1")
                for s in range(4):
                    first = True
                    for h in range(2):
                        off = (0 if h == 0 else J0) + int(Kg[i, h, :s].sum())
                        for cidx in range(int(Kg[i, h, s])):
                            j = off + cidx
                            last = (h == 1 and cidx == int(Kg[i, 1, s]) - 1)
                            nc.tensor.matmul(
                                out=psum_t1[32 * s:32 * s + 32, :],
                                lhsT=wm[:, j:j + 1, :],
                                rhs=xg[:, j:j + 1, :],
                                start=first, stop=last,
                                tile_position=(0, 32 * s))
                            first = False

                t1sb = sb.tile([128, WC], f32, tag="t1sb")
                nc.scalar.copy(out=t1sb[:], in_=psum_t1[:])
                xb = sb.tile([128, WC], f32, tag="xb")
                nc.sync.dma_start(out=xb[:], in_=xslot.ap()[i * P:(i + 1) * P, :])

                xt = sb.tile([128, WC], f32, tag="xt")
                t1t = sb.tile([128, WC], f32, tag="t1t")
                for t in range(3):
                    ptr = pstr.tile([128, 128], f32, space="PSUM", tag="tr")
                    nc.tensor.transpose(out=ptr[:], in_=xb[:, 128 * t:128 * t + 128],
                                        identity=id_t[:])
                    nc.vector.tensor_copy(out=xt[:, 128 * t:128 * t + 128], in_=ptr[:])
                    ptr2 = pstr.tile([128, 128], f32, space="PSUM", tag="tr")
                    nc.tensor.transpose(out=ptr2[:], in_=t1sb[:, 128 * t:128 * t + 128],
                                        identity=id_t[:])
                    nc.scalar.copy(out=t1t[:, 128 * t:128 * t + 128], in_=ptr2[:])

                yo = sb.tile([128, WC], f32, tag="yo")
                osb = sb.tile([128, WC], f32, tag="osb")
                for go in range(3):
                    py = psy.tile([128, 128], f32, space="PSUM", tag="y")
                    plist = pairs_by_go[go]
                    for n_, (pi_, gi, path) in enumerate(plist):
                        rhs = (xt if path == 0 else t1t)[:, 128 * gi:128 * gi + 128]
                        nc.tensor.matmul(
                            out=py[:], lhsT=mats_t[:, 128 * pi_:128 * pi_ + 128],
                            rhs=rhs, start=(n_ == 0), stop=(n_ == len(plist) - 1),
                            tile_position=(0, 0))
                    ysl = yo[:, 128 * go:128 * go + 128]
                    nc.scalar.activation(out=ysl, in_=py[:],
                                         func=mybir.ActivationFunctionType.Identity,
                                         bias=bias_t[:, go:go + 1], scale=1.0)
                    tl = sb.tile([128, 128], f32, tag="tl")
                    nc.vector.tensor_scalar_mul(out=tl[:], in0=ysl, scalar1=0.01)
                    nc.vector.tensor_tensor(out=ysl, in0=ysl, in1=tl[:],
                                            op=mybir.AluOpType.max)
                    ptr3 = pstr.tile([128, 128], f32, space="PSUM", tag="tr")
                    nc.tensor.transpose(out=ptr3[:], in_=ysl, identity=id_t[:])
                    nc.vector.tensor_copy(out=osb[:, 128 * go:128 * go + 128],
                                          in_=ptr3[:])
                nc.sync.dma_start(out=out_pc.ap()[i * P:(i + 1) * P, :], in_=osb[:])

    nc.compile()
    return nc


def kernel(x, A, Ew, Wcheb, bcheb, Wconv, bconv, batch_size=1):
    from concourse.bass_utils import run_bass_kernel_spmd

    xrow16, xslot, idx16, dstl_t, what_t, Kg, Jh, Ji, joff, JT, IWT = \
        _host_prep(x, A, Ew)
    mats_sb, bias_sb, pairs = _fold_weights(Wcheb, bcheb, Wconv, bconv)

    key = (JT, IWT, tuple(Ji.tolist()))
    if key not in _cache:
        _cache[key] = _build_program(Kg, Jh, Ji, joff, JT, IWT, len(pairs))
    nc = _cache[key]

    iota_np = np.tile(np.arange(32, dtype=np.float16)[None, :], (128, 1))
    ident_np = np.eye(128, dtype=np.float32)
    in_maps = []
    for c in range(NCORES):
        in_maps.append(dict(
            xrow16=xrow16, xslot=xslot[c], idx16=idx16[c],
            dstl=dstl_t[c], what=what_t[c], mats=mats_sb, biasd=bias_sb,
            iota=iota_np, ident=ident_np))
    res = run_bass_kernel_spmd(nc, in_maps, core_ids=list(range(NCORES)))
    full = np.concatenate([res.results[c]["out_pc"] for c in range(NCORES)], axis=0)
    return np.ascontiguousarray(full[:N]).reshape(N, W, C).astype(np.float32)



# revision 2
# speedup vs baseline: 1.8907x; 1.8907x over previous
"""ChebConv (K=2) + temporal Conv1d GNN kernel for 8 Trainium2 NeuronCores.

Strategy (data-parallel over destination nodes, channel-major on chip):
  - Node axis padded to 50176 = 392 blocks of 128; core c owns blocks
    [49c, 49c+49).
  - Host precomputes w_hat (edge weights of -D^-1/2 A D^-1/2), quantizes x
    to fp8-e4m3 rows padded to 512B (descriptor-efficient gathers), and
    sorts the edge list by (dst block, src half, dst subblock-of-32) with
    16-aligned group sizes shared across cores (max over cores).
  - Per block the device gathers fp8 source rows with SWDGE dma_gather,
    builds 32-wide one-hot*w_hat fp8 masks on DVE, and aggregates messages
    with TensorE matmuls that keep the result CHANNEL-major (x rows are the
    stationary operand), so no on-chip transposes are needed anywhere.
  - Chebyshev combine + temporal conv collapse into 14 dense 128x128
    matmuls per block with host-prefolded fp16 weights (x^T streamed from
    HBM); LeakyReLU finishes on-chip; fp16 channel-major output is
    de-transposed on the host.
"""

import numpy as np
import ml_dtypes

N = 50000
E = 1600000
W = 12
C = 32
WC = W * C            # 384
NCORES = 8
P = 128
NPAD = 50176          # 392 * 128
NB = NPAD // P        # 392
SLOTS = NB // NCORES  # 49
HALF = NPAD // 2      # 25088
GELEM = 512           # fp8 row bytes (384 data + 128 pad)
XS = 8.0              # x fp8 scale
WS = 64.0             # w_hat fp8 scale
DS = 1.0 / (XS * WS)
F8NP = ml_dtypes.float8_e4m3

_cache = {}


def _host_prep(x, A, Ew):
    src = np.asarray(A[0], np.int64)
    dst = np.asarray(A[1], np.int64)
    Ew = np.asarray(Ew, np.float32)

    deg = np.bincount(dst, weights=Ew.astype(np.float64), minlength=N).astype(np.float32)
    dinv = np.where(deg > 0, 1.0 / np.sqrt(np.maximum(deg, 1e-12)), 0.0).astype(np.float32)
    w_hat = (-dinv[src] * Ew * dinv[dst]).astype(np.float32)

    xn = np.asarray(x, np.float32).transpose(1, 0, 2).reshape(N, WC)
    xrow8 = np.zeros((NPAD, GELEM), F8NP)
    xrow8[:N, :WC] = np.clip(xn * XS, -224.0, 224.0).astype(F8NP)
    xpad = np.zeros((NPAD, WC), np.float16)
    xpad[:N] = xn
    # channel-major x for the cheb-fold matmuls: [128, 3, NPAD]
    xT = np.ascontiguousarray(xpad.T.reshape(3, 128, NPAD).transpose(1, 0, 2))

    blk = dst >> 7
    hh = (src >= HALF).astype(np.int64)
    sb = (dst >> 5) & 3
    gid = (blk * 2 + hh) * 4 + sb
    order = np.argsort(gid, kind="stable")
    loc = (src[order] - hh[order] * HALF).astype(np.int16)
    dl = (dst[order] & 127).astype(np.float16)
    wv = np.clip(w_hat[order] * WS, -224.0, 224.0).astype(np.float16)

    counts = np.bincount(gid, minlength=NB * 8)
    gstart = np.zeros(NB * 8 + 1, np.int64)
    np.cumsum(counts, out=gstart[1:])
    ccore = counts.reshape(NCORES, SLOTS, 2, 4)
    NU = ((ccore.max(axis=0) + 15) // 16 * 16).astype(np.int64)   # [49, 2, 4]
    NU = np.maximum(NU, 16)
    OFF = np.zeros((SLOTS, 2, 5), np.int64)
    np.cumsum(NU, axis=2, out=OFF[:, :, 1:])
    NUH = OFF[:, :, 4]                                            # [49, 2]
    JH = -(-NUH // 128)                                           # [49, 2]

    ioff_flat = np.zeros(SLOTS * 2 + 1, np.int64)
    np.cumsum((NUH // 16).reshape(-1), out=ioff_flat[1:])
    IOFF = ioff_flat[:-1].reshape(SLOTS, 2)
    ITOT = int(ioff_flat[-1])
    coff_flat = np.zeros(SLOTS * 2 + 1, np.int64)
    np.cumsum(JH.reshape(-1), out=coff_flat[1:])
    CF = coff_flat[:-1].reshape(SLOTS, 2)
    JCOLTOT = int(coff_flat[-1])

    idx16 = np.zeros((NCORES, 128, ITOT), np.int16)
    dstl = np.full((NCORES, 128, JCOLTOT), 255.0, np.float16)
    what = np.zeros((NCORES, 128, JCOLTOT), np.float16)

    for c in range(NCORES):
        for i in range(SLOTS):
            for h in range(2):
                nuh = int(NUH[i, h])
                jh = int(JH[i, h])
                V = np.zeros(nuh, np.int16)
                D = np.full(jh * 128, 255.0, np.float16)
                Wv = np.zeros(jh * 128, np.float16)
                for s in range(4):
                    g = (((c * SLOTS + i) * 2 + h) * 4 + s)
                    n = int(counts[g])
                    sl = slice(int(gstart[g]), int(gstart[g]) + n)
                    o = int(OFF[i, h, s])
                    V[o:o + n] = loc[sl]
                    D[o:o + n] = dl[sl]
                    Wv[o:o + n] = wv[sl]
                io = int(IOFF[i, h])
                idx16[c, :, io:io + nuh // 16] = np.tile(V.reshape(-1, 16).T, (8, 1))
                co = int(CF[i, h])
                dstl[c, :, co:co + jh] = D.reshape(jh, 128).T
                what[c, :, co:co + jh] = Wv.reshape(jh, 128).T

    return (xrow8, xT, idx16, dstl, what, w_hat,
            NU, OFF, NUH, JH, IOFF, CF, ITOT, JCOLTOT)


def _fold_weights(Wcheb, bcheb, Wconv, bconv):
    Wcheb = np.asarray(Wcheb, np.float32)
    bcheb = np.asarray(bcheb, np.float32)
    Wconv = np.asarray(Wconv, np.float32)
    bconv = np.asarray(bconv, np.float32)
    pairs = []
    for go in range(3):
        for gi in range(max(0, go - 1), min(3, go + 2)):
            for path in range(2):
                pairs.append((path, gi, go))
    mats = np.zeros((len(pairs), 128, 128), np.float32)
    for pi, (path, gi, go) in enumerate(pairs):
        for wo in range(4 * go, 4 * go + 4):
            for k in range(3):
                wi = wo + k - 1
                if not (4 * gi <= wi < 4 * gi + 4) or not (0 <= wi < W):
                    continue
                Cmat = Wcheb[wi, path] @ Wconv[:, :, k].T  # [ci, co]
                r0 = 32 * (wi - 4 * gi)
                c0 = 32 * (wo - 4 * go)
                mats[pi, r0:r0 + 32, c0:c0 + 32] = Cmat
    mats_sb = np.ascontiguousarray(
        mats.transpose(1, 0, 2).reshape(128, -1)).astype(np.float16)
    bias = np.zeros((12, 32), np.float32)
    for wo in range(12):
        bias[wo] = bconv.copy()
        for k in range(3):
            wi = wo + k - 1
            if 0 <= wi < W:
                bias[wo] += bcheb[wi] @ Wconv[:, :, k].T
    bias_sb = bias.reshape(3, 128).T.copy()  # [128, 3]
    return mats_sb, bias_sb, pairs


def _build_program(NU, OFF, NUH, JH, IOFF, CF, ITOT, JCOLTOT, n_pairs):
    import concourse.bacc as bacc
    import concourse.tile as tile
    from concourse import mybir

    nc = bacc.Bacc("TRN2", target_bir_lowering=False, debug=False,
                   num_devices=NCORES)
    f16, f32, i16 = mybir.dt.float16, mybir.dt.float32, mybir.dt.int16
    f8 = mybir.dt.float8e4
    xrow8 = nc.dram_tensor("xrow8", [NPAD, GELEM], f8, kind="ExternalInput")
    xTd = nc.dram_tensor("xT", [128, 3, SLOTS * P], f16, kind="ExternalInput")
    idxd = nc.dram_tensor("idx16", [128, ITOT], i16, kind="ExternalInput")
    dstld = nc.dram_tensor("dstl", [128, JCOLTOT], f16, kind="ExternalInput")
    whatd = nc.dram_tensor("what", [128, JCOLTOT], f16, kind="ExternalInput")
    matsd = nc.dram_tensor("mats", [128, n_pairs * 128], f16, kind="ExternalInput")
    biasd = nc.dram_tensor("biasd", [128, 3], f32, kind="ExternalInput")
    iotad = nc.dram_tensor("iota", [128, 128], f16, kind="ExternalInput")
    out_pc = nc.dram_tensor("out_pc", [128, 3, SLOTS * P], f16, kind="ExternalOutput")

    pairs_by_go = [[], [], []]
    pi = 0
    for go in range(3):
        for gi in range(max(0, go - 1), min(3, go + 2)):
            for path in range(2):
                pairs_by_go[go].append((pi, gi, path))
                pi += 1

    JSMAX = int((JH[:, 0] + JH[:, 1]).max())
    # per-slot (h, s) one-hot column ranges
    WMX = 0
    WSMAX = 0
    for i in range(SLOTS):
        wtot = 0
        for h in range(2):
            for s in range(4):
                o0, o1 = int(OFF[i, h, s]), int(OFF[i, h, s] + NU[i, h, s] - 1)
                wn = o1 // 128 - o0 // 128 + 1
                wtot += wn
                WSMAX = max(WSMAX, wn)
        WMX = max(WMX, wtot)

    with tile.TileContext(nc) as tc:
        with tc.tile_pool(name="const", bufs=1) as cp, \
             tc.tile_pool(name="xgp", bufs=2) as xgp, \
             tc.tile_pool(name="wmp", bufs=2) as wmp, \
             tc.tile_pool(name="eqp", bufs=2) as eqp, \
             tc.tile_pool(name="t1p", bufs=2) as t1p, \
             tc.tile_pool(name="tlp", bufs=2) as tlp, \
             tc.tile_pool(name="pst1", bufs=2, space="PSUM") as pst1, \
             tc.tile_pool(name="psy", bufs=2, space="PSUM") as psy:
            mats_t = cp.tile([128, n_pairs * 128], f16)
            nc.sync.dma_start(out=mats_t[:], in_=matsd.ap())
            bias_t = cp.tile([128, 3], f32)
            nc.sync.dma_start(out=bias_t[:], in_=biasd.ap())
            iota_t = cp.tile([128, 128], f16)
            nc.sync.dma_start(out=iota_t[:], in_=iotad.ap())
            idx_t = cp.tile([128, ITOT], i16)
            nc.sync.dma_start(out=idx_t[:], in_=idxd.ap())
            dm_t = cp.tile([128, JCOLTOT], f16)
            nc.sync.dma_start(out=dm_t[:], in_=dstld.ap())
            wh_t = cp.tile([128, JCOLTOT], f16)
            nc.sync.dma_start(out=wh_t[:], in_=whatd.ap())
            xt_t = cp.tile([128, 3, SLOTS * P], f16)
            nc.sync.dma_start(out=xt_t[:], in_=xTd.ap())
            stage_t = cp.tile([128, 3, SLOTS * P], f16)
            zero3 = cp.tile([128, 3, 128], f8)
            nc.vector.memset(zero3[:], 0.0)

            # prime the gather buffers so chunk-tail padding reads are finite
            for _ in range(2):
                t = xgp.tile([128, JSMAX, GELEM], f8, tag="xg")
                nc.vector.memset(t[:], 0.0)

            for i in range(SLOTS):
                xg_t = xgp.tile([128, JSMAX, GELEM], f8, tag="xg")
                for h in range(2):
                    ch = 0 if h == 0 else int(JH[i, 0])
                    nuh = int(NUH[i, h])
                    io = int(IOFF[i, h])
                    nc.gpsimd.dma_gather(
                        xg_t[:, ch:ch + int(JH[i, h]), :],
                        xrow8.ap()[h * HALF:(h + 1) * HALF, :],
                        idx_t[:, io:io + nuh // 16],
                        nuh, nuh, GELEM,
                        single_packet=False)

                # one-hot * w_hat masks (fp8), one region per (h, s)
                wm_t = wmp.tile([128, WMX, 32], f8, tag="wm")
                woff = 0
                mmlist = []  # (xg col, wm col, s)
                for h in range(2):
                    chb = 0 if h == 0 else int(JH[i, 0])
                    for s in range(4):
                        o0 = int(OFF[i, h, s])
                        o1 = o0 + int(NU[i, h, s]) - 1
                        c0, c1 = o0 // 128, o1 // 128
                        wn = c1 - c0 + 1
                        a = int(CF[i, h]) + c0
                        eq_t = eqp.tile([128, WSMAX, 32], f16, tag="eq")
                        nc.vector.tensor_tensor(
                            out=eq_t[:, :wn, :],
                            in0=dm_t[:, a:a + wn].unsqueeze(2).to_broadcast([128, wn, 32]),
                            in1=iota_t[:, 32 * s:32 * s + 32].unsqueeze(1).to_broadcast([128, wn, 32]),
                            op=mybir.AluOpType.is_equal)
                        nc.vector.tensor_tensor(
                            out=wm_t[:, woff:woff + wn, :],
                            in0=eq_t[:, :wn, :],
                            in1=wh_t[:, a:a + wn].unsqueeze(2).to_broadcast([128, wn, 32]),
                            op=mybir.AluOpType.mult)
                        for q in range(wn):
                            mmlist.append((chb + c0 + q, woff + q, s))
                        woff += wn

                # message aggregation, channel-major: t1T[ch, dst] in PSUM
                pst = pst1.tile([128, 3, 128], f32, space="PSUM", tag="t1")
                nc.tensor.matmul(out=pst[:], lhsT=zero3[:, 0, :], rhs=zero3[:],
                                 start=True, stop=False, skip_group_check=True)
                total = 3 * len(mmlist)
                k = 0
                for b in range(3):
                    for (xcol, wcol, s) in mmlist:
                        k += 1
                        nc.tensor.matmul(
                            out=pst[:, b, 32 * s:32 * s + 32],
                            lhsT=xg_t[:, xcol, b * 128:(b + 1) * 128],
                            rhs=wm_t[:, wcol, :],
                            start=False, stop=(k == total),
                            skip_group_check=True)

                t1sb = t1p.tile([128, 3, 128], f16, tag="t1sb")
                nc.scalar.mul(out=t1sb[:], in_=pst[:], mul=DS)

                # cheb + temporal-conv fold (channel-major y)
                yps = psy.tile([128, 3, 128], f32, space="PSUM", tag="y")
                for go in range(3):
                    plist = pairs_by_go[go]
                    for n_, (pi_, gi, path) in enumerate(plist):
                        rhs = (xt_t[:, gi, i * P:(i + 1) * P] if path == 0
                               else t1sb[:, gi, :])
                        nc.tensor.matmul(
                            out=yps[:, go, :],
                            lhsT=mats_t[:, pi_ * 128:(pi_ + 1) * 128],
                            rhs=rhs,
                            start=(n_ == 0), stop=(n_ == len(plist) - 1),
                            skip_group_check=True)
                    ysl = stage_t[:, go, i * P:(i + 1) * P]
                    nc.scalar.activation(out=ysl, in_=yps[:, go, :],
                                         func=mybir.ActivationFunctionType.Identity,
                                         bias=bias_t[:, go:go + 1], scale=1.0)
                    tl = tlp.tile([128, 128], f16, tag="tl")
                    nc.vector.tensor_scalar_mul(out=tl[:], in0=ysl, scalar1=0.01)
                    nc.vector.tensor_tensor(out=ysl, in0=ysl, in1=tl[:],
                                            op=mybir.AluOpType.max)

            nc.sync.dma_start(out=out_pc.ap(), in_=stage_t[:])

    nc.compile()
    return nc


def kernel(x, A, Ew, Wcheb, bcheb, Wconv, bconv, batch_size=1):
    from concourse.bass_utils import run_bass_kernel_spmd

    (xrow8, xT, idx16, dstl, what, w_hat,
     NU, OFF, NUH, JH, IOFF, CF, ITOT, JCOLTOT) = _host_prep(x, A, Ew)
    mats_sb, bias_sb, pairs = _fold_weights(Wcheb, bcheb, Wconv, bconv)

    key = (ITOT, JCOLTOT, tuple(NU.reshape(-1).tolist()))
    if key not in _cache:
        _cache[key] = _build_program(NU, OFF, NUH, JH, IOFF, CF,
                                     ITOT, JCOLTOT, len(pairs))
    nc = _cache[key]

    iota_np = np.tile(np.arange(128, dtype=np.float16)[None, :], (128, 1))
    in_maps = []
    for c in range(NCORES):
        in_maps.append(dict(
            xrow8=xrow8,
            xT=np.ascontiguousarray(xT[:, :, c * SLOTS * P:(c + 1) * SLOTS * P]),
            idx16=idx16[c], dstl=dstl[c], what=what[c],
            mats=mats_sb, biasd=bias_sb, iota=iota_np))
    res = run_bass_kernel_spmd(nc, in_maps, core_ids=list(range(NCORES)))
    # out_pc[c]: [128, 3, 6272] channel-major fp16 -> [50000, 12, 32] f32
    cols = [np.asarray(res.results[c]["out_pc"], np.float16).reshape(128, 3, SLOTS * P)
            for c in range(NCORES)]
    full = np.concatenate(cols, axis=2)                  # [128, 3, 50176]
    full = full.transpose(1, 0, 2).reshape(WC, NPAD)     # [384, 50176]
    y = np.ascontiguousarray(full[:, :N].T).astype(np.float32)  # [N, 384]
    return y.reshape(N, W, C)


# revision 5
# speedup vs baseline: 1.9451x; 1.0288x over previous
"""ChebConv (K=2) + temporal Conv1d GNN kernel for 8 Trainium2 NeuronCores.

Strategy (data-parallel over destination nodes, channel-major on chip):
  - Node axis padded to 50176 = 392 blocks of 128; core c owns blocks
    [49c, 49c+49).
  - Host precomputes w_hat (edge weights of -D^-1/2 A D^-1/2), quantizes x
    to fp8-e4m3 rows padded to 512B (descriptor-efficient gathers), and
    sorts the edge list by (dst block, src half, dst subblock-of-32) with
    16-aligned group sizes shared across cores (max over cores).
  - Per block the device gathers fp8 source rows with SWDGE dma_gather,
    builds 32-wide one-hot*w_hat fp8 masks on DVE, and aggregates messages
    with TensorE matmuls that keep the result CHANNEL-major (x rows are the
    stationary operand), so no on-chip transposes are needed anywhere.
  - Chebyshev combine + temporal conv collapse into 14 dense 128x128
    matmuls per block with host-prefolded fp16 weights (x^T streamed from
    HBM); LeakyReLU finishes on-chip; fp16 channel-major output is
    de-transposed on the host.
"""

import numpy as np
import ml_dtypes

N = 50000
E = 1600000
W = 12
C = 32
WC = W * C            # 384
NCORES = 8
P = 128
NPAD = 50176          # 392 * 128
NB = NPAD // P        # 392
SLOTS = NB // NCORES  # 49
HALF = NPAD // 2      # 25088
GELEM = 512           # fp8 row bytes (384 data + 128 pad)
XS = 8.0              # x fp8 scale
WS = 64.0             # w_hat fp8 scale
DS = 1.0 / (XS * WS)
F8NP = ml_dtypes.float8_e4m3

_cache = {}


def _host_prep(x, A, Ew):
    src = np.asarray(A[0], np.int64)
    dst = np.asarray(A[1], np.int64)
    Ew = np.asarray(Ew, np.float32)

    deg = np.bincount(dst, weights=Ew.astype(np.float64), minlength=N).astype(np.float32)
    dinv = np.where(deg > 0, 1.0 / np.sqrt(np.maximum(deg, 1e-12)), 0.0).astype(np.float32)
    w_hat = (-dinv[src] * Ew * dinv[dst]).astype(np.float32)

    xn = np.asarray(x, np.float32).transpose(1, 0, 2).reshape(N, WC)
    xrow8 = np.zeros((NPAD, GELEM), F8NP)
    xrow8[:N, :WC] = np.clip(xn * XS, -224.0, 224.0).astype(F8NP)
    xpad = np.zeros((NPAD, WC), np.float16)
    xpad[:N] = xn
    # channel-major x for the cheb-fold matmuls: [128, 3, NPAD]
    xT = np.ascontiguousarray(xpad.T.reshape(3, 128, NPAD).transpose(1, 0, 2))

    blk = dst >> 7
    hh = (src >= HALF).astype(np.int64)
    sb = (dst >> 5) & 3
    gid = (blk * 2 + hh) * 4 + sb
    order = np.argsort(gid, kind="stable")
    loc = (src[order] - hh[order] * HALF).astype(np.int16)
    dl = (dst[order] & 127).astype(np.float16)
    wv = np.clip(w_hat[order] * WS, -224.0, 224.0).astype(np.float16)

    counts = np.bincount(gid, minlength=NB * 8)
    gstart = np.zeros(NB * 8 + 1, np.int64)
    np.cumsum(counts, out=gstart[1:])
    ccore = counts.reshape(NCORES, SLOTS, 2, 4)
    NU = ccore.max(axis=0).astype(np.int64)                       # [49, 2, 4]
    OFF = np.zeros((SLOTS, 2, 5), np.int64)
    np.cumsum(NU, axis=2, out=OFF[:, :, 1:])
    NUH = (OFF[:, :, 4] + 15) // 16 * 16                          # [49, 2]
    JH = -(-NUH // 128)                                           # [49, 2]

    ioff_flat = np.zeros(SLOTS * 2 + 1, np.int64)
    np.cumsum((NUH // 16).reshape(-1), out=ioff_flat[1:])
    IOFF = ioff_flat[:-1].reshape(SLOTS, 2)
    ITOT = int(ioff_flat[-1])
    coff_flat = np.zeros(SLOTS * 2 + 1, np.int64)
    np.cumsum(JH.reshape(-1), out=coff_flat[1:])
    CF = coff_flat[:-1].reshape(SLOTS, 2)
    JCOLTOT = int(coff_flat[-1])

    idx16 = np.zeros((NCORES, 128, ITOT), np.int16)
    dstl = np.full((NCORES, 128, JCOLTOT), 255.0, np.float16)
    what = np.zeros((NCORES, 128, JCOLTOT), np.float16)

    for c in range(NCORES):
        for i in range(SLOTS):
            for h in range(2):
                nuh = int(NUH[i, h])
                jh = int(JH[i, h])
                V = np.zeros(nuh, np.int16)
                D = np.full(jh * 128, 255.0, np.float16)
                Wv = np.zeros(jh * 128, np.float16)
                for s in range(4):
                    g = (((c * SLOTS + i) * 2 + h) * 4 + s)
                    n = int(counts[g])
                    sl = slice(int(gstart[g]), int(gstart[g]) + n)
                    o = int(OFF[i, h, s])
                    V[o:o + n] = loc[sl]
                    D[o:o + n] = dl[sl]
                    Wv[o:o + n] = wv[sl]
                io = int(IOFF[i, h])
                idx16[c, :, io:io + nuh // 16] = np.tile(V.reshape(-1, 16).T, (8, 1))
                co = int(CF[i, h])
                dstl[c, :, co:co + jh] = D.reshape(jh, 128).T
                what[c, :, co:co + jh] = Wv.reshape(jh, 128).T

    return (xrow8, xT, idx16, dstl, what, w_hat,
            NU, OFF, NUH, JH, IOFF, CF, ITOT, JCOLTOT)


def _fold_weights(Wcheb, bcheb, Wconv, bconv):
    Wcheb = np.asarray(Wcheb, np.float32)
    bcheb = np.asarray(bcheb, np.float32)
    Wconv = np.asarray(Wconv, np.float32)
    bconv = np.asarray(bconv, np.float32)
    pairs = []
    for go in range(3):
        for gi in range(max(0, go - 1), min(3, go + 2)):
            for path in range(2):
                pairs.append((path, gi, go))
    mats = np.zeros((len(pairs), 128, 128), np.float32)
    for pi, (path, gi, go) in enumerate(pairs):
        for wo in range(4 * go, 4 * go + 4):
            for k in range(3):
                wi = wo + k - 1
                if not (4 * gi <= wi < 4 * gi + 4) or not (0 <= wi < W):
                    continue
                Cmat = Wcheb[wi, path] @ Wconv[:, :, k].T  # [ci, co]
                r0 = 32 * (wi - 4 * gi)
                c0 = 32 * (wo - 4 * go)
                mats[pi, r0:r0 + 32, c0:c0 + 32] = Cmat
    mats_sb = np.ascontiguousarray(
        mats.transpose(1, 0, 2).reshape(128, -1)).astype(np.float16)
    bias = np.zeros((12, 32), np.float32)
    for wo in range(12):
        bias[wo] = bconv.copy()
        for k in range(3):
            wi = wo + k - 1
            if 0 <= wi < W:
                bias[wo] += bcheb[wi] @ Wconv[:, :, k].T
    bias_sb = bias.reshape(3, 128).T.copy()  # [128, 3]
    return mats_sb, bias_sb, pairs


def _build_program(NU, OFF, NUH, JH, IOFF, CF, ITOT, JCOLTOT, n_pairs):
    import concourse.bacc as bacc
    import concourse.tile as tile
    from concourse import mybir

    nc = bacc.Bacc("TRN2", target_bir_lowering=False, debug=False,
                   num_devices=NCORES)
    f16, f32, i16 = mybir.dt.float16, mybir.dt.float32, mybir.dt.int16
    f8 = mybir.dt.float8e4
    xrow8 = nc.dram_tensor("xrow8", [NPAD, GELEM], f8, kind="ExternalInput")
    xTd = nc.dram_tensor("xT", [128, 3, SLOTS * P], f16, kind="ExternalInput")
    idxd = nc.dram_tensor("idx16", [128, ITOT], i16, kind="ExternalInput")
    dstld = nc.dram_tensor("dstl", [128, JCOLTOT], f16, kind="ExternalInput")
    whatd = nc.dram_tensor("what", [128, JCOLTOT], f16, kind="ExternalInput")
    matsd = nc.dram_tensor("mats", [128, n_pairs * 128], f16, kind="ExternalInput")
    biasd = nc.dram_tensor("biasd", [128, 3], f32, kind="ExternalInput")
    iotad = nc.dram_tensor("iota", [128, 128], f16, kind="ExternalInput")
    out_pc = nc.dram_tensor("out_pc", [128, 3, SLOTS * P], f16, kind="ExternalOutput")

    pairs_by_go = [[], [], []]
    pi = 0
    for go in range(3):
        for gi in range(max(0, go - 1), min(3, go + 2)):
            for path in range(2):
                pairs_by_go[go].append((pi, gi, path))
                pi += 1

    JSMAX = int((JH[:, 0] + JH[:, 1]).max())
    # per-slot (h, s) one-hot column ranges
    WMX = 0
    WSMAX = 0
    for i in range(SLOTS):
        wtot = 0
        for h in range(2):
            for s in range(4):
                if NU[i, h, s] == 0:
                    continue
                o0, o1 = int(OFF[i, h, s]), int(OFF[i, h, s] + NU[i, h, s] - 1)
                wn = o1 // 128 - o0 // 128 + 1
                wtot += wn
                WSMAX = max(WSMAX, wn)
        WMX = max(WMX, wtot)

    with tile.TileContext(nc) as tc:
        with tc.tile_pool(name="const", bufs=1) as cp, \
             tc.tile_pool(name="xgp", bufs=2) as xgp, \
             tc.tile_pool(name="wmp", bufs=2) as wmp, \
             tc.tile_pool(name="eqp", bufs=2) as eqp, \
             tc.tile_pool(name="t1p", bufs=2) as t1p, \
             tc.tile_pool(name="tlp", bufs=2) as tlp, \
             tc.tile_pool(name="stp", bufs=2) as stp, \
             tc.tile_pool(name="pst1", bufs=2, space="PSUM") as pst1, \
             tc.tile_pool(name="psy", bufs=2, space="PSUM") as psy:
            # idx first: slot-0 gathers only depend on it, so the big x^T /
            # weight loads queue behind the first gathers instead of
            # delaying them.
            idx_t = cp.tile([128, ITOT], i16)
            nc.sync.dma_start(out=idx_t[:], in_=idxd.ap())
            zero3 = cp.tile([128, 3, 128], f8)
            nc.vector.memset(zero3[:], 0.0)
            mats_t = cp.tile([128, n_pairs * 128], f16)
            bias_t = cp.tile([128, 3], f32)
            iota_t = cp.tile([128, 128], f16)
            dm_t = cp.tile([128, JCOLTOT], f16)
            wh_t = cp.tile([128, JCOLTOT], f16)
            xt_t = cp.tile([128, 3, SLOTS * P], f16)

            stage_t = None
            for i in range(SLOTS):
                xg_t = xgp.tile([128, JSMAX, GELEM], f8, tag="xg")
                for h in range(2):
                    ch = 0 if h == 0 else int(JH[i, 0])
                    nuh = int(NUH[i, h])
                    io = int(IOFF[i, h])
                    if nuh % 128:
                        # last gather column is only partially written; zero
                        # it so chunk-tail reads stay finite
                        nc.vector.memset(
                            xg_t[:, ch + int(JH[i, h]) - 1, :], 0.0)
                    nc.gpsimd.dma_gather(
                        xg_t[:, ch:ch + int(JH[i, h]), :],
                        xrow8.ap()[h * HALF:(h + 1) * HALF, :],
                        idx_t[:, io:io + nuh // 16],
                        nuh, nuh, GELEM,
                        single_packet=False)
                if i == 0:
                    nc.sync.dma_start(out=dm_t[:], in_=dstld.ap())
                    nc.sync.dma_start(out=wh_t[:], in_=whatd.ap())
                    nc.sync.dma_start(out=iota_t[:], in_=iotad.ap())
                    nc.sync.dma_start(out=mats_t[:], in_=matsd.ap())
                    nc.sync.dma_start(out=bias_t[:], in_=biasd.ap())
                    nc.sync.dma_start(out=xt_t[:], in_=xTd.ap())
                if i % 7 == 0:
                    stage_t = stp.tile([128, 3, 7 * P], f16, tag="st")

                # one-hot * w_hat masks (fp8), one region per (h, s)
                wm_t = wmp.tile([128, WMX, 32], f8, tag="wm")
                woff = 0
                mmlist = []  # (xg col, wm col, s)
                for h in range(2):
                    chb = 0 if h == 0 else int(JH[i, 0])
                    for s in range(4):
                        if NU[i, h, s] == 0:
                            continue
                        o0 = int(OFF[i, h, s])
                        o1 = o0 + int(NU[i, h, s]) - 1
                        c0, c1 = o0 // 128, o1 // 128
                        wn = c1 - c0 + 1
                        a = int(CF[i, h]) + c0
                        eq_t = eqp.tile([128, WSMAX, 32], f16, tag="eq")
                        nc.vector.tensor_tensor(
                            out=eq_t[:, :wn, :],
                            in0=dm_t[:, a:a + wn].unsqueeze(2).to_broadcast([128, wn, 32]),
                            in1=iota_t[:, 32 * s:32 * s + 32].unsqueeze(1).to_broadcast([128, wn, 32]),
                            op=mybir.AluOpType.is_equal)
                        nc.vector.tensor_tensor(
                            out=wm_t[:, woff:woff + wn, :],
                            in0=eq_t[:, :wn, :],
                            in1=wh_t[:, a:a + wn].unsqueeze(2).to_broadcast([128, wn, 32]),
                            op=mybir.AluOpType.mult)
                        for q in range(wn):
                            mmlist.append((chb + c0 + q, woff + q, s))
                        woff += wn

                # message aggregation, channel-major: t1T[ch, dst] in PSUM
                pst = pst1.tile([128, 3, 128], f32, space="PSUM", tag="t1")
                nc.tensor.matmul(out=pst[:], lhsT=zero3[:, 0, :], rhs=zero3[:],
                                 start=True, stop=False, skip_group_check=True)
                total = 3 * len(mmlist)
                k = 0
                for b in range(3):
                    for (xcol, wcol, s) in mmlist:
                        k += 1
                        nc.tensor.matmul(
                            out=pst[:, b, 32 * s:32 * s + 32],
                            lhsT=xg_t[:, xcol, b * 128:(b + 1) * 128],
                            rhs=wm_t[:, wcol, :],
                            start=False, stop=(k == total),
                            skip_group_check=True)

                t1sb = t1p.tile([128, 3, 128], f16, tag="t1sb")
                nc.scalar.mul(out=t1sb[:], in_=pst[:], mul=DS)

                # cheb + temporal-conv fold (channel-major y)
                yps = psy.tile([128, 3, 128], f32, space="PSUM", tag="y")
                for go in range(3):
                    plist = pairs_by_go[go]
                    for n_, (pi_, gi, path) in enumerate(plist):
                        rhs = (xt_t[:, gi, i * P:(i + 1) * P] if path == 0
                               else t1sb[:, gi, :])
                        nc.tensor.matmul(
                            out=yps[:, go, :],
                            lhsT=mats_t[:, pi_ * 128:(pi_ + 1) * 128],
                            rhs=rhs,
                            start=(n_ == 0), stop=(n_ == len(plist) - 1),
                            skip_group_check=True)
                    ysl = stage_t[:, go, (i % 7) * P:(i % 7 + 1) * P]
                    nc.scalar.activation(out=ysl, in_=yps[:, go, :],
                                         func=mybir.ActivationFunctionType.Identity,
                                         bias=bias_t[:, go:go + 1], scale=1.0)
                    tl = tlp.tile([128, 128], f16, tag="tl")
                    nc.vector.tensor_scalar_mul(out=tl[:], in0=ysl, scalar1=0.01)
                    nc.vector.tensor_tensor(out=ysl, in0=ysl, in1=tl[:],
                                            op=mybir.AluOpType.max)

                if i % 7 == 6:
                    nc.sync.dma_start(
                        out=out_pc.ap()[:, :, (i - 6) * P:(i + 1) * P],
                        in_=stage_t[:])

    nc.compile()
    return nc


def kernel(x, A, Ew, Wcheb, bcheb, Wconv, bconv, batch_size=1):
    from concourse.bass_utils import run_bass_kernel_spmd

    (xrow8, xT, idx16, dstl, what, w_hat,
     NU, OFF, NUH, JH, IOFF, CF, ITOT, JCOLTOT) = _host_prep(x, A, Ew)
    mats_sb, bias_sb, pairs = _fold_weights(Wcheb, bcheb, Wconv, bconv)

    key = (ITOT, JCOLTOT, tuple(NU.reshape(-1).tolist()))
    if key not in _cache:
        _cache[key] = _build_program(NU, OFF, NUH, JH, IOFF, CF,
                                     ITOT, JCOLTOT, len(pairs))
    nc = _cache[key]

    iota_np = np.tile(np.arange(128, dtype=np.float16)[None, :], (128, 1))
    in_maps = []
    for c in range(NCORES):
        in_maps.append(dict(
            xrow8=xrow8,
            xT=np.ascontiguousarray(xT[:, :, c * SLOTS * P:(c + 1) * SLOTS * P]),
            idx16=idx16[c], dstl=dstl[c], what=what[c],
            mats=mats_sb, biasd=bias_sb, iota=iota_np))
    res = run_bass_kernel_spmd(nc, in_maps, core_ids=list(range(NCORES)))
    # out_pc[c]: [128, 3, 6272] channel-major fp16 -> [50000, 12, 32] f32
    cols = [np.asarray(res.results[c]["out_pc"], np.float16).reshape(128, 3, SLOTS * P)
            for c in range(NCORES)]
    full = np.concatenate(cols, axis=2)                  # [128, 3, 50176]
    full = full.transpose(1, 0, 2).reshape(WC, NPAD)     # [384, 50176]
    y = np.ascontiguousarray(full[:, :N].T).astype(np.float32)  # [N, 384]
    return y.reshape(N, W, C)
